# revision 1
# baseline (speedup 1.0000x reference)
"""Trainium2 Bass kernel for nn_DotProductScorer.

Computes, for ragged candidate tokens split into B segments by `starts`:
    q  = state_vec @ Wq.T + bq          [B, d_token]
    kq = q @ Wk.T                       [B, d_token]
    logits[i] = dot(cand_tokens[i], kq[seg(i)])   for each token i
with tokens outside [starts[0], starts[-1]) zeroed.

Sharding: cand_tokens (and the per-token segment mapping) are sharded along
the token axis K across 8 NeuronCores; the small Wq/bq/Wk params (and the
per-core slice of state_vec needed for the local kq table) ride along.  Each
core computes its kq rows on-device (two small PE matmuls), then streams its
128 MiB cand shard through:
  - one DVE tensor_tensor multiply per chunk (kq operand broadcast along the
    free axis with a stride-0 AP), in place, then
  - a per-token reduction split between DVE tensor_reduce (first NR tokens of
    each chunk) and ACT activation-with-accum (the rest), so the two engines
    finish together just under the HBM-stream time.

Fast path (uniform starts, SEG=512 — what reference.setup_inputs produces):
tokens are laid out segment-major: partition p of group g handles segment
g*128+p, so the kq operand of every multiply is a [128,128] slice of the
resident kq table — no gather, no broadcast across partitions, and both the
input and output DMAs are contiguous per partition.

General path (any sorted `starts`): host derives per-token segment ids
(index bookkeeping, as per the sharding hint) and expands the kq table to a
per-token E = kq[seg] array; each core streams cand and E shards through the
same multiply + split-reduction loop.
"""

import numpy as np

import concourse.bass as bass
import concourse.tile as tile
from concourse import bacc, mybir
from concourse.bass_utils import run_bass_kernel_spmd

B = 4096
SEG = 512
K = B * SEG
D_STATE = 256
D_TOKEN = 128
NCORES = 8
SEGS_PER_CORE = B // NCORES           # 512
TOK_PER_CORE = K // NCORES            # 262144

F32 = mybir.dt.float32
AF = mybir.ActivationFunctionType
ALU = mybir.AluOpType
AX = mybir.AxisListType


def _reduce_chunk(nc, ch, L, obase, och, nr):
    """Per-token dots from the product chunk ch [128, och, 128] into
    L[:, obase:obase+och]: first `nr` tokens via one DVE tensor_reduce,
    the rest via ACT activation-accumulate (one op per token, writing its
    pass-through output in place so consecutive ACT ops don't WAW-serialize
    on a shared scratch)."""
    if nr > 0:
        nc.vector.tensor_reduce(out=L[:, obase:obase + nr],
                                in_=ch[:, 0:nr, :], axis=AX.X, op=ALU.add)
    for j in range(nr, och):
        nc.scalar.activation(ch[:, j, :], ch[:, j, :], AF.Copy,
                             bias=0.0, scale=1.0,
                             accum_out=L[:, obase + j:obase + j + 1])


def build_fast(segs_per_core=SEGS_PER_CORE, seg=SEG, och=64, nr=42,
               chunk_bufs=3, n_rep=1):
    """Uniform-starts program. Per core:
      inputs : svT [256, S] (state rows for this core's S segments, transposed)
               WqT [256,128], WkT [128,128], bq [128,1], cand [S*seg, 128]
      output : out [S*seg] f32
    Token layout: group g (128 segments), partition p = segment g*128+p,
    free index o in [0, seg) -> local token (g*128+p)*seg + o.
    """
    groups = segs_per_core // 128
    assert segs_per_core % 128 == 0 and seg % och == 0
    nchunk = seg // och
    tok = segs_per_core * seg

    nc = bacc.Bacc("TRN2", target_bir_lowering=False, debug=False,
                   num_devices=NCORES)
    svT = nc.dram_tensor("svT", [D_STATE, segs_per_core], F32,
                         kind="ExternalInput").ap()
    WqT = nc.dram_tensor("WqT", [D_STATE, D_TOKEN], F32,
                         kind="ExternalInput").ap()
    WkT = nc.dram_tensor("WkT", [D_TOKEN, D_TOKEN], F32,
                         kind="ExternalInput").ap()
    bqv = nc.dram_tensor("bq", [D_TOKEN, 1], F32, kind="ExternalInput").ap()
    cand = nc.dram_tensor("cand", [tok, D_TOKEN], F32,
                          kind="ExternalInput").ap()
    out = nc.dram_tensor("out", [tok], F32, kind="ExternalOutput").ap()

    cand_r = cand.rearrange("(g p o) d -> g p o d", g=groups, p=128, o=seg)
    out_r = out.rearrange("(g p o) -> g p o", g=groups, p=128, o=seg)

    with tile.TileContext(nc) as tc:
        with (
            tc.tile_pool(name="const", bufs=1) as constp,
            tc.tile_pool(name="psum", bufs=2, space="PSUM") as psump,
            tc.tile_pool(name="chunk", bufs=chunk_bufs) as chunkp,
            tc.tile_pool(name="lout", bufs=2) as loutp,
        ):
            # ---- prologue: kq = (sv @ Wq.T + bq) @ Wk.T, segment-major ----
            svT_t = constp.tile([128, 2, segs_per_core], F32)
            nc.sync.dma_start(svT_t[:, 0, :], svT[0:128, :])
            nc.sync.dma_start(svT_t[:, 1, :], svT[128:256, :])
            WqT_t = constp.tile([128, 2, D_TOKEN], F32)
            nc.sync.dma_start(WqT_t[:, 0, :], WqT[0:128, :])
            nc.sync.dma_start(WqT_t[:, 1, :], WqT[128:256, :])
            WkT_t = constp.tile([128, D_TOKEN], F32)
            nc.sync.dma_start(WkT_t[:], WkT[:])
            bq_t = constp.tile([128, 1], F32)
            nc.sync.dma_start(bq_t[:], bqv[:])

            # qT[d_tok, s] = sum_ds Wq[d_tok, ds] * sv[s, ds]
            qT_sb = constp.tile([128, segs_per_core], F32)
            for h in range(0, segs_per_core, 512):
                w = min(512, segs_per_core - h)
                qT_ps = psump.tile([128, 512], F32, tag="qT_ps")
                nc.tensor.matmul(qT_ps[:, :w], WqT_t[:, 0, :],
                                 svT_t[:, 0, h:h + w], start=True, stop=False)
                nc.tensor.matmul(qT_ps[:, :w], WqT_t[:, 1, :],
                                 svT_t[:, 1, h:h + w], start=False, stop=True)
                # + bq (per-partition bias) while copying PSUM -> SBUF
                nc.scalar.activation(qT_sb[:, h:h + w], qT_ps[:, :w],
                                     AF.Identity, bias=bq_t[:], scale=1.0)

            # kq[s, d2] = sum_d1 qT[d1, s] * WkT[d1, d2]; partition = segment
            kq_sb = constp.tile([128, groups, D_TOKEN], F32)
            for g in range(groups):
                kq_ps = psump.tile([128, D_TOKEN], F32, tag="kq_ps")
                nc.tensor.matmul(kq_ps[:], qT_sb[:, g * 128:(g + 1) * 128],
                                 WkT_t[:], start=True, stop=True)
                nc.scalar.copy(kq_sb[:, g, :], kq_ps[:])

            # ---- main: multiply + split reduction over the cand stream ----
            # n_rep > 1 re-runs the stream over the same data (bench only).
            for _rep in range(n_rep):
                for g in range(groups):
                    L = loutp.tile([128, seg], F32)
                    kq_b = kq_sb[:, g, :].unsqueeze(1).broadcast_to(
                        [128, och, D_TOKEN])
                    for kk in range(nchunk):
                        ch = chunkp.tile([128, och, D_TOKEN], F32)
                        nc.sync.dma_start(ch[:],
                                          cand_r[g, :, kk * och:(kk + 1) * och, :])
                        nc.vector.tensor_tensor(out=ch[:], in0=ch[:], in1=kq_b,
                                                op=ALU.mult)
                        _reduce_chunk(nc, ch, L, kk * och, och, nr)
                    nc.sync.dma_start(out_r[g, :, :], L[:])

    nc.compile()
    return nc


def build_general(tok_per_core=TOK_PER_CORE, och=64, nr=42, chunk_bufs=3):
    """Any-starts program. Per core:
      inputs : cand [T, 128], E [T, 128] (host-gathered kq[seg] rows,
               zeroed outside the valid range)
      output : out [T] f32
    Token layout: partition p handles tokens p*(T/128) .. (p+1)*(T/128).
    """
    assert tok_per_core % (128 * och) == 0
    a_len = tok_per_core // 128
    nchunk = a_len // och

    nc = bacc.Bacc("TRN2", target_bir_lowering=False, debug=False,
                   num_devices=NCORES)
    cand = nc.dram_tensor("cand", [tok_per_core, D_TOKEN], F32,
                          kind="ExternalInput").ap()
    ev = nc.dram_tensor("E", [tok_per_core, D_TOKEN], F32,
                        kind="ExternalInput").ap()
    out = nc.dram_tensor("out", [tok_per_core], F32,
                         kind="ExternalOutput").ap()

    cand_r = cand.rearrange("(p a) d -> p a d", p=128, a=a_len)
    e_r = ev.rearrange("(p a) d -> p a d", p=128, a=a_len)
    out_r = out.rearrange("(p a) -> p a", p=128, a=a_len)

    with tile.TileContext(nc) as tc:
        with (
            tc.tile_pool(name="chunk", bufs=chunk_bufs) as chunkp,
            tc.tile_pool(name="echunk", bufs=chunk_bufs) as echunkp,
            tc.tile_pool(name="lout", bufs=1) as loutp,
        ):
            L = loutp.tile([128, a_len], F32)
            for kk in range(nchunk):
                ch = chunkp.tile([128, och, D_TOKEN], F32)
                nc.sync.dma_start(ch[:], cand_r[:, kk * och:(kk + 1) * och, :])
                eh = echunkp.tile([128, och, D_TOKEN], F32)
                nc.sync.dma_start(eh[:], e_r[:, kk * och:(kk + 1) * och, :])
                nc.vector.tensor_tensor(out=ch[:], in0=ch[:], in1=eh[:],
                                        op=ALU.mult)
                _reduce_chunk(nc, ch, L, kk * och, och, nr)
            nc.sync.dma_start(out_r[:, :], L[:])

    nc.compile()
    return nc


_PROG_CACHE = {}


def _get_prog(kind):
    if kind not in _PROG_CACHE:
        _PROG_CACHE[kind] = build_fast() if kind == "fast" else build_general()
    return _PROG_CACHE[kind]


def _is_uniform(starts):
    if starts.shape != (B + 1,):
        return False
    return bool(np.array_equal(starts.astype(np.int64),
                               np.arange(B + 1, dtype=np.int64) * SEG))


def fast_in_maps(state_vec, cand_tokens, Wq, bq, Wk):
    WqT = np.ascontiguousarray(Wq.T)                 # [256, 128]
    WkT = np.ascontiguousarray(Wk.T)                 # [128, 128]
    bq2 = np.ascontiguousarray(bq.reshape(D_TOKEN, 1))
    in_maps = []
    for c in range(NCORES):
        svT_c = np.ascontiguousarray(
            state_vec[c * SEGS_PER_CORE:(c + 1) * SEGS_PER_CORE].T)
        cand_c = cand_tokens[c * TOK_PER_CORE:(c + 1) * TOK_PER_CORE]
        in_maps.append({"svT": svT_c, "WqT": WqT, "WkT": WkT,
                        "bq": bq2, "cand": cand_c})
    return in_maps


def kernel(state_vec, cand_tokens, starts, Wq, bq, Wk):
    state_vec = np.ascontiguousarray(np.asarray(state_vec, dtype=np.float32))
    cand_tokens = np.ascontiguousarray(np.asarray(cand_tokens, dtype=np.float32))
    starts = np.asarray(starts)
    Wq = np.ascontiguousarray(np.asarray(Wq, dtype=np.float32))
    bq = np.ascontiguousarray(np.asarray(bq, dtype=np.float32))
    Wk = np.ascontiguousarray(np.asarray(Wk, dtype=np.float32))

    core_ids = list(range(NCORES))
    if _is_uniform(starts):
        nc = _get_prog("fast")
        in_maps = fast_in_maps(state_vec, cand_tokens, Wq, bq, Wk)
        res = run_bass_kernel_spmd(nc, in_maps, core_ids)
        return np.concatenate([res.results[c]["out"] for c in core_ids])

    # ---- general path: host derives seg ids / expands kq (index work) ----
    nc = _get_prog("general")
    idx = np.arange(K, dtype=np.int64)
    s64 = starts.astype(np.int64)
    seg = np.searchsorted(s64, idx, side="right") - 1
    seg = np.clip(seg, 0, B - 1)
    valid = (idx >= s64[0]) & (idx < s64[-1])
    kq = ((state_vec @ Wq.T + bq) @ Wk.T).astype(np.float32)
    E = kq[seg]
    E[~valid] = 0.0
    in_maps = []
    for c in range(NCORES):
        in_maps.append({
            "cand": cand_tokens[c * TOK_PER_CORE:(c + 1) * TOK_PER_CORE],
            "E": np.ascontiguousarray(E[c * TOK_PER_CORE:(c + 1) * TOK_PER_CORE]),
        })
    res = run_bass_kernel_spmd(nc, in_maps, core_ids)
    return np.concatenate([res.results[c]["out"] for c in core_ids])

